# revision 1
# baseline (speedup 1.0000x reference)

# CRGCN multi-behavior GCN forward loss on 8 Trainium2 NeuronCores.
#
# Strategy (graph/data parallel, dest-node sharding):
#  - Nodes (users+items, 200000 -> padded 200704) are sharded row-wise across
#    8 cores (25088 = 196*128 nodes/core). Edges are partitioned by the shard
#    of their destination (col) node on the host, bucketed by (128-dest tile,
#    source bucket of 28672 rows) and padded so every 128-edge chunk maps to
#    one dest tile and one source bucket. The chunk schedule is the max over
#    cores so a single SPMD program fits all 8 cores.
#  - Per behavior each core holds a bf16 table T2 = [dinv*total | total]
#    ([200704, 128], 256B rows) for ALL nodes, produced by AllGather of
#    per-shard slabs. Message pass: dma_gather (int16 in-bucket indices) of
#    T2 rows for edge sources; a 0/1 one-hot (edge x dest-in-tile) built on
#    DVE from edge cols; PE matmul contracts edges, accumulating
#    S^T[feat, dest] = sum_e dinv[r_e]*total[r_e] x onehot in PSUM per dest
#    tile; then S @ W, *dinv[d], +b, l2-normalize, residual-accumulate into
#    the SBUF-resident fp32 total shard.
#  - deg (in-degree) is a one-hot x ones matmul (bf16, exact), per behavior,
#    from the same col data.
#  - BPR loss: batch rows sharded across cores; u/pos/neg rows fetched with
#    per-partition indirect DMA from the raw-total half of T2; dots +
#    softplus(-d) (relu + log1p poly) on-device; partials AllGathered so all
#    cores emit the identical final scalar.

import sys

sys.path.insert(0, "/opt/trn_rl_repo")

import dataclasses
import numpy as np

# ---------------- problem constants (hardcoded; kernel.py is standalone) ---
N_USERS = 100000
N_ITEMS = 100000
N_NODES = 200000
EMBED = 64
N_BEH = 3
BATCH = 4096
REG_WEIGHT = 1e-4
NCORES = 8

FULL_CFG = dict(
    ncores=NCORES,
    embed=EMBED,
    nbeh=N_BEH,
    shard=25088,          # 196 * 128
    nt=196,               # dest tiles per shard
    wt=128,               # T2 row width in bf16 elems (256B)
    nbuck=7,              # source buckets
    bucket=28672,         # rows per bucket (7 * 28672 = 200704)
    wtiles=8,             # dest tiles per gather window
    g=32,                 # chunks per one-hot build group
    flush=14,             # tiles per T2 staging flush (196 = 14*14)
    batch=BATCH,
    batch_per_core=BATCH // NCORES,   # 512
    n_nodes=N_NODES,
    reg_weight=REG_WEIGHT,
)


# ---------------------------------------------------------------------------
# Host-side preprocessing
# ---------------------------------------------------------------------------
def make_schedule_and_arrays(edges, cfg):
    """edges: [NB, 2, E]. Builds the (window, bucket, tile)-ordered common
    chunk schedule and the per-core col/idx arrays."""
    ncores = cfg["ncores"]
    NT = cfg["nt"]
    NB = cfg["nbeh"]
    NBK = cfg["nbuck"]
    BUCK = cfg["bucket"]
    WT = cfg["wtiles"]
    NW = (NT + WT - 1) // WT

    sched = {"C": [], "cells": [], "tiles": [], "windows": [],
             "tile_cstart": []}
    cols_arr = [[None] * NB for _ in range(ncores)]
    idx_arr = [[None] * NB for _ in range(ncores)]

    for b in range(NB):
        row = np.asarray(edges[b, 0], dtype=np.int64)
        col = np.asarray(edges[b, 1], dtype=np.int64)
        gt = col >> 7                       # global dest tile
        s_of = gt // NT                     # owning core
        t_of = gt - s_of * NT               # local dest tile
        beta = row // BUCK                  # source bucket
        # per (core, tile, bucket) counts
        cellkey = (s_of * NT + t_of) * NBK + beta
        cnt = np.bincount(cellkey, minlength=ncores * NT * NBK).reshape(
            ncores, NT, NBK)
        K_cell = -(-cnt.max(axis=0) // 128)           # [NT, NBK]
        empty_t = K_cell.sum(axis=1) == 0
        K_cell[empty_t, 0] = 1

        # gather order: (window, bucket, tile); consumption order:
        # (window, tile, bucket). Chunks get positions in both orders.
        C = int(K_cell.sum())
        cell_start = {}      # gather-order chunk start per cell
        cell_cstart = {}     # consumption-order chunk start per cell
        pos = 0
        for w in range(NW):
            ts = range(w * WT, min((w + 1) * WT, NT))
            for be in range(NBK):
                for t in ts:
                    if K_cell[t, be]:
                        cell_start[(t, be)] = pos
                        pos += int(K_cell[t, be])
        assert pos == C
        cpos = 0
        tile_cstart = np.zeros(NT + 1, dtype=np.int64)
        for w in range(NW):
            ts = range(w * WT, min((w + 1) * WT, NT))
            for t in ts:
                tile_cstart[t] = cpos
                for be in range(NBK):
                    if K_cell[t, be]:
                        cell_cstart[(t, be)] = cpos
                        cpos += int(K_cell[t, be])
        tile_cstart[NT] = cpos
        assert cpos == C

        # per-tile consumption: ordered chunk positions + total K per tile
        tiles = []
        for t in range(NT):
            plist = []
            for be in range(NBK):
                if K_cell[t, be]:
                    st = cell_start[(t, be)]
                    plist.extend(range(st, st + int(K_cell[t, be])))
            tiles.append(plist)

        # per-window gather segments: (bucket, pos_start, n_chunks)
        windows = []
        for w in range(NW):
            ts = range(w * WT, min((w + 1) * WT, NT))
            segs = []
            for be in range(NBK):
                n = int(sum(K_cell[t, be] for t in ts))
                if n:
                    st = min(cell_start[(t, be)] for t in ts
                             if K_cell[t, be])
                    segs.append((be, st, n))
            windows.append(segs)

        sched["C"].append(C)
        sched["cells"].append((K_cell, cell_start))
        sched["tiles"].append(tiles)
        sched["windows"].append(windows)
        sched["tile_cstart"].append(tile_cstart)

        # ------------- per-core arrays -------------
        starts_np = np.zeros((NT, NBK), dtype=np.int64)
        for (t, be), st in cell_start.items():
            starts_np[t, be] = st
        cstarts_np = np.zeros((NT, NBK), dtype=np.int64)
        for (t, be), st in cell_cstart.items():
            cstarts_np[t, be] = st
        for s in range(ncores):
            colv = np.full(C * 128, 128.0, dtype=np.float32)
            rowv = np.zeros(C * 128, dtype=np.int64)   # in-bucket idx
            sel = s_of == s
            r_s = row[sel]
            c_s = col[sel]
            t_s = t_of[sel]
            be_s = beta[sel]
            key = t_s * NBK + be_s
            order = np.argsort(key, kind="stable")
            r_s, c_s, t_s, be_s, key = (r_s[order], c_s[order], t_s[order],
                                        be_s[order], key[order])
            seg_start = np.searchsorted(key, np.arange(NT * NBK))
            within = np.arange(len(key)) - seg_start[key]
            dst = starts_np[t_s, be_s] * 128 + within
            cdst = cstarts_np[t_s, be_s] * 128 + within
            colv[cdst] = (c_s & 127).astype(np.float32)
            rowv[dst] = r_s - be_s * BUCK
            import ml_dtypes as _md
            cols_arr[s][b] = np.ascontiguousarray(
                colv.reshape(C, 128).T).astype(_md.bfloat16)   # [128, C]
            # idx16: [128, C*8]; gather element i -> [i%16 (+16k), off+i//16]
            iv = rowv.reshape(C * 128)
            i16 = np.zeros((16, C * 8), dtype=np.int16)
            ii = np.arange(C * 128)
            i16[ii % 16, ii // 16] = iv.astype(np.int16)
            idx_arr[s][b] = np.ascontiguousarray(np.tile(i16, (8, 1)))

    return sched, cols_arr, idx_arr


def make_inputs_per_core(inputs, cfg, sched_arrays):
    import ml_dtypes

    ncores = cfg["ncores"]
    SH = cfg["shard"]
    E = cfg["embed"]
    NB = cfg["nbeh"]
    BPC = cfg["batch_per_core"]
    BJ = BPC // 128
    n_nodes = cfg["n_nodes"]
    n_users = n_nodes // 2

    sched, cols_arr, idx_arr = sched_arrays

    user_emb = np.asarray(inputs["user_emb"], dtype=np.float32)
    item_emb = np.asarray(inputs["item_emb"], dtype=np.float32)
    gcn_weight = np.asarray(inputs["gcn_weight"], dtype=np.float32)
    gcn_bias = np.asarray(inputs["gcn_bias"], dtype=np.float32)
    batch_data = np.asarray(inputs["batch_data"], dtype=np.int64)

    total0 = np.concatenate([user_emb, item_emb], axis=0)

    iota = np.tile(np.arange(128, dtype=np.float32)[None, :],
                   (128, 1)).astype(ml_dtypes.bfloat16)
    w_bf = gcn_weight.astype(ml_dtypes.bfloat16)
    bb = np.tile(gcn_bias[:, None, :], (1, 128, 1)).astype(np.float32)

    in_maps = []
    for s in range(ncores):
        lo = s * SH
        hi = min((s + 1) * SH, n_nodes)
        init_shard = np.zeros((SH, E), dtype=np.float32)
        if hi > lo:
            init_shard[: hi - lo] = total0[lo:hi]

        bidx = np.zeros((NB * 3, 128, BJ), dtype=np.int32)
        rs = slice(s * BPC, (s + 1) * BPC)
        for b in range(NB):
            u = batch_data[rs, b, 0].astype(np.int32)
            p = batch_data[rs, b, 1].astype(np.int32) + n_users
            n = batch_data[rs, b, 2].astype(np.int32) + n_users
            for k, v in enumerate((u, p, n)):
                bidx[b * 3 + k] = v.reshape(BJ, 128).T

        m = {
            "init_shard": init_shard,
            "iota_in": iota,
            "w_in": w_bf,
            "bb_in": bb,
            "bidx_in": bidx,
        }
        for b in range(NB):
            m[f"col{b}"] = cols_arr[s][b]
            m[f"idx{b}"] = idx_arr[s][b]
        in_maps.append(m)
    return in_maps


# ---------------------------------------------------------------------------
# Device program
# ---------------------------------------------------------------------------
def build_program(cfg, sched):
    from concourse import bass, bacc, mybir, tile

    dt = mybir.dt
    AF = mybir.ActivationFunctionType
    ALU = mybir.AluOpType

    ncores = cfg["ncores"]
    NT = cfg["nt"]
    SH = cfg["shard"]
    NTOT = SH * ncores
    E = cfg["embed"]
    WT = cfg["wt"]            # 128 table cols
    NBK = cfg["nbuck"]
    BUCK = cfg["bucket"]
    WTL = cfg["wtiles"]
    G = cfg["g"]
    FLUSH = cfg["flush"]
    BPC = cfg["batch_per_core"]
    BJ = BPC // 128
    NB = cfg["nbeh"]
    NV = NB + 1
    NW = (NT + WTL - 1) // WTL

    C = sched["C"]
    tiles_md = sched["tiles"]
    windows_md = sched["windows"]
    cstart_md = sched["tile_cstart"]

    # max chunks in any window (for the staging tile size)
    wch_max = 0
    for b in range(NB):
        for w in range(NW):
            wch = sum(n for (_, _, n) in windows_md[b][w])
            wch_max = max(wch_max, wch)

    def bc(ap, where, n):
        newap = list(ap.ap)
        newap.insert(where, [0, n])
        return dataclasses.replace(ap, ap=newap)

    nc = bacc.Bacc("TRN2", target_bir_lowering=False, debug=False,
                   num_devices=ncores, num_swdge_queues=4)

    f32, bf16, i32, i16 = dt.float32, dt.bfloat16, dt.int32, dt.int16
    shared = "Local"

    init_in = nc.dram_tensor("init_shard", [SH, E], f32,
                             kind="ExternalInput").ap()
    iota_in = nc.dram_tensor("iota_in", [128, 128], bf16,
                             kind="ExternalInput").ap()
    w_in = nc.dram_tensor("w_in", [NB, E, E], bf16, kind="ExternalInput").ap()
    bb_in = nc.dram_tensor("bb_in", [NB, 128, E], f32,
                           kind="ExternalInput").ap()
    bidx_in = nc.dram_tensor("bidx_in", [NB * 3, 128, BJ], i32,
                             kind="ExternalInput").ap()
    col_in = [nc.dram_tensor(f"col{b}", [128, C[b]], bf16,
                             kind="ExternalInput").ap() for b in range(NB)]
    idx_in = [nc.dram_tensor(f"idx{b}", [128, C[b] * 8], i16,
                             kind="ExternalInput").ap() for b in range(NB)]
    loss_out = nc.dram_tensor("loss", [1, 1], f32, kind="ExternalOutput").ap()

    with tile.TileContext(nc) as tc:
        with (
            tc.tile_pool(name="dram", bufs=1, space="DRAM") as dpool,
            tc.tile_pool(name="pers", bufs=1) as pers,
            tc.tile_pool(name="work", bufs=2) as work,
            tc.tile_pool(name="small", bufs=4) as small,
            tc.tile_pool(name="ppx", bufs=2, space="PSUM") as ppx,
            tc.tile_pool(name="ppy", bufs=2, space="PSUM") as ppy,
            tc.tile_pool(name="ppd", bufs=2, space="PSUM") as ppd,
        ):
            t2s = [dpool.tile([SH, WT], bf16, tag=f"t2s{v}",
                              name=f"t2s{v}") for v in range(NV)]
            t2f = [dpool.tile([NTOT, WT], bf16, tag=f"t2f{v}",
                              name=f"t2f{v}", addr_space=shared)
                   for v in range(NV)]
            lag_i = dpool.tile([1, 2], f32, tag="lag_i", name="lag_i")
            lag_o = dpool.tile([ncores, 2], f32, tag="lag_o", name="lag_o",
                               addr_space=shared)

            tot = pers.tile([128, NT * E], f32, tag="tot", name="tot")
            iota = pers.tile([128, 128], bf16, tag="iota", name="iota")
            wsb = pers.tile([E, NB * E], bf16, tag="wsb", name="wsb")
            bbsb = pers.tile([128, NB * E], f32, tag="bbsb", name="bbsb")
            bidx = pers.tile([128, NB * 3 * BJ], i32, tag="bidx", name="bidx")
            deg = [pers.tile([128, NT], f32, tag=f"deg{b}", name=f"deg{b}")
                   for b in range(NB)]
            dinv = [pers.tile([128, NT], f32, tag=f"dinv{v}", name=f"dinv{v}")
                    for v in range(NV)]
            onesb = pers.tile([128, 1], bf16, tag="onesb", name="onesb")
            onesf = pers.tile([128, 1], f32, tag="onesf", name="onesf")
            racc = pers.tile([128, 16], f32, tag="racc", name="racc")
            blacc = pers.tile([128, NB], f32, tag="blacc", name="blacc")

            nc.sync.dma_start(out=iota[:], in_=iota_in)
            nc.sync.dma_start(
                out=wsb[:].rearrange("k (b e) -> k b e", b=NB),
                in_=w_in.rearrange("b k e -> k b e"))
            nc.sync.dma_start(
                out=bbsb[:].rearrange("p (b e) -> p b e", b=NB),
                in_=bb_in.rearrange("b p e -> p b e"))
            nc.sync.dma_start(
                out=bidx[:].rearrange("p (a j) -> p a j", a=NB * 3),
                in_=bidx_in.rearrange("a p j -> p a j"))
            nc.sync.dma_start(
                out=tot[:].rearrange("p (t e) -> p t e", e=E),
                in_=init_in.rearrange("(t p) e -> p t e", p=128))
            nc.vector.memset(onesb[:], 1.0)
            nc.vector.memset(onesf[:], 1.0)
            nc.vector.memset(dinv[NB][:], 0.0)

            # reg term: sum of squares of the initial embeddings
            NREG = (NT * E + 1023) // 1024
            sqd = pers.tile([128, 1024], f32, tag="sqd", name="sqd")
            for i in range(NREG):
                sl = slice(i * 1024, min((i + 1) * 1024, NT * E))
                nc.scalar.activation(out=sqd[:, : sl.stop - sl.start],
                                     in_=tot[:, sl], func=AF.Square,
                                     accum_out=racc[:, i:i + 1])

            # ------- lazy consumption-ordered one-hot group builder -------
            class IndBuilder:
                """Builds one-hot groups for consumption positions
                [c0, c0+wch) on demand, in order, so only a few groups are
                live at once."""

                def __init__(self, cs, c0, wch):
                    self.cs, self.c0, self.wch = cs, c0, wch
                    self.groups = {}

                def get(self, cpos):
                    rel = cpos - self.c0
                    g0 = (rel // G) * G
                    if g0 not in self.groups:
                        gw = min(G, self.wch - g0)
                        ind = work.tile([128, G * 128], dt.bfloat16,
                                        tag="ind", name="ind", bufs=3)
                        iv = ind[:].rearrange("p (g d) -> p g d", d=128)
                        nc.vector.tensor_tensor(
                            out=iv[:, :gw, :],
                            in0=bc(iota[:], 1, gw),
                            in1=bc(self.cs[:, g0:g0 + gw], 2, 128),
                            op=ALU.is_equal)
                        self.groups[g0] = ind
                    return self.groups[g0], rel - g0

            # ---------------- deg pass ----------------
            def deg_pass(b):
                for w in range(NW):
                    segs = windows_md[b][w]
                    if not segs:
                        continue
                    wch = sum(n for (_, _, n) in segs)
                    t0w = w * WTL
                    c0 = int(cstart_md[b][t0w])
                    cs = small.tile([128, wch_max], bf16, tag="csd",
                                    name="csd", bufs=3)
                    nc.sync.dma_start(out=cs[:, :wch],
                                      in_=col_in[b][:, c0:c0 + wch])
                    bld = IndBuilder(cs, c0, wch)
                    for t in range(t0w, min((t0w + WTL), NT)):
                        plist = tiles_md[b][t]
                        dg_ps = ppd.tile([128, 1], f32, tag="dg", name="dg",
                                         bufs=3)
                        for j in range(len(plist)):
                            ind, r = bld.get(int(cstart_md[b][t]) + j)
                            nc.tensor.matmul(
                                out=dg_ps[:],
                                lhsT=ind[:, r * 128:(r + 1) * 128],
                                rhs=onesb[:],
                                start=(j == 0), stop=(j == len(plist) - 1))
                        nc.scalar.copy(out=deg[b][:, t:t + 1], in_=dg_ps[:])
                d1 = small.tile([128, NT], f32, tag="d1", name="d1")
                d2 = small.tile([128, NT], f32, tag="d2", name="d2")
                nc.vector.tensor_scalar(out=d1[:], in0=deg[b][:], scalar1=1.0,
                                        scalar2=None, op0=ALU.max)
                nc.scalar.activation(out=d1[:], in_=d1[:], func=AF.Sqrt)
                nc.vector.reciprocal(out=d1[:], in_=d1[:])
                nc.vector.tensor_scalar(out=d2[:], in0=deg[b][:], scalar1=0.0,
                                        scalar2=None, op0=ALU.is_gt)
                nc.vector.tensor_tensor(out=dinv[b][:], in0=d1[:], in1=d2[:],
                                        op=ALU.mult)

            # ------------- T2 assemble sweep + AllGather -------------
            def assemble(v):
                for tf in range(0, NT, FLUSH):
                    n = min(FLUSH, NT - tf)
                    s65 = work.tile([128, FLUSH * WT], bf16, tag="s65",
                                    name="s65")
                    for i in range(n):
                        t = tf + i
                        totsl = tot[:, t * E:(t + 1) * E]
                        nc.vector.tensor_scalar(
                            out=s65[:, i * WT:i * WT + E], in0=totsl,
                            scalar1=dinv[v][:, t:t + 1], scalar2=None,
                            op0=ALU.mult)
                        nc.scalar.copy(
                            out=s65[:, i * WT + E:i * WT + 2 * E], in_=totsl)
                    nc.sync.dma_start(
                        out=t2s[v][:].rearrange(
                            "(t p) w -> p t w", p=128)[:, tf:tf + n, :],
                        in_=s65[:, :n * WT].rearrange(
                            "p (t w) -> p t w", w=WT))
                nc.gpsimd.collective_compute(
                    "AllGather", ALU.bypass,
                    replica_groups=[list(range(ncores))],
                    ins=[t2s[v][:].opt()], outs=[t2f[v][:].opt()])

            # ---------------- main pass ----------------
            def main_pass(b):
                for w in range(NW):
                    segs = windows_md[b][w]
                    g0 = min(st for (_, st, _) in segs)
                    wch = sum(n for (_, _, n) in segs)
                    t0w = w * WTL
                    c0 = int(cstart_md[b][t0w])
                    cs = small.tile([128, wch_max], bf16, tag="cs", name="cs",
                                    bufs=3)
                    nc.sync.dma_start(out=cs[:, :wch],
                                      in_=col_in[b][:, c0:c0 + wch])
                    ixs = small.tile([128, wch_max * 8], i16, tag="ixs",
                                     name="ixs", bufs=2)
                    nc.sync.dma_start(out=ixs[:, :wch * 8],
                                      in_=idx_in[b][:, g0 * 8:(g0 + wch) * 8])
                    gat = work.tile([128, wch_max * 128], bf16, tag="gat",
                                    name="gat")
                    gv = gat[:].rearrange("p (c e) -> p c e", e=128)
                    for (be, st, n) in segs:
                        for o in range(0, n, 8):
                            m = min(8, n - o)
                            so = st - g0 + o
                            nc.gpsimd.dma_gather(
                                out_ap=gv[:, so:so + m, :],
                                in_ap=t2f[b][be * BUCK:(be + 1) * BUCK, :],
                                idxs_ap=ixs[:, so * 8:(so + m) * 8],
                                num_idxs=m * 128,
                                num_idxs_reg=m * 128,
                                elem_size=WT)
                    bld = IndBuilder(cs, c0, wch)
                    for t in range(t0w, min(t0w + WTL, NT)):
                        plist = tiles_md[b][t]
                        xt_ps = ppx.tile([E, 128], f32, tag="xt", name="xt")
                        for j, pos in enumerate(plist):
                            ind, r = bld.get(int(cstart_md[b][t]) + j)
                            nc.tensor.matmul(
                                out=xt_ps[:],
                                lhsT=gv[:, pos - g0, 0:E],
                                rhs=ind[:, r * 128:(r + 1) * 128],
                                start=(j == 0), stop=(j == len(plist) - 1))
                        post_tile(b, t, xt_ps)

            def post_tile(b, t, xt_ps):
                xts = small.tile([E, 128], bf16, tag="xts", name="xts")
                nc.vector.tensor_copy(out=xts[:], in_=xt_ps[:])
                y_ps = ppy.tile([128, E], f32, tag="y", name="y")
                nc.tensor.matmul(out=y_ps[:], lhsT=xts[:],
                                 rhs=wsb[:, b * E:(b + 1) * E],
                                 start=True, stop=True)
                z = small.tile([128, E], f32, tag="z", name="z")
                nc.vector.tensor_scalar(out=z[:], in0=y_ps[:],
                                        scalar1=dinv[b][:, t:t + 1],
                                        scalar2=None, op0=ALU.mult)
                nc.vector.tensor_tensor(out=z[:], in0=z[:],
                                        in1=bbsb[:, b * E:(b + 1) * E],
                                        op=ALU.add)
                sq = small.tile([128, E], f32, tag="sq", name="sq")
                ss = small.tile([128, 1], f32, tag="ss", name="ss")
                nc.scalar.activation(out=sq[:], in_=z[:], func=AF.Square,
                                     accum_out=ss[:])
                nc.scalar.activation(out=ss[:], in_=ss[:], func=AF.Sqrt)
                nc.vector.tensor_scalar(out=ss[:], in0=ss[:], scalar1=1e-12,
                                        scalar2=None, op0=ALU.max)
                rin = small.tile([128, 1], f32, tag="rin", name="rin")
                nc.vector.reciprocal(out=rin[:], in_=ss[:])
                nc.vector.tensor_scalar(out=z[:], in0=z[:], scalar1=rin[:, :],
                                        scalar2=None, op0=ALU.mult)
                totsl = tot[:, t * E:(t + 1) * E]
                nc.vector.tensor_tensor(out=totsl, in0=totsl, in1=z[:],
                                        op=ALU.add)

            # ---------------- loss ----------------
            LOG1P_C = [2.4139025189026897e-09, 0.9999996692324197,
                       -0.499988759640371, 0.3331669190104936,
                       -0.2486582066434577, 0.19337637102999028,
                       -0.14517645896753417, 0.09470379566439587,
                       -0.04713346504062944, 0.015145372148722138,
                       -0.002288060381570317]

            def loss_pass(b):
                gs = []
                for k in range(3):
                    gk = small.tile([128, BJ * WT], bf16, tag=f"bg{k}",
                                    name=f"bg{k}")
                    gkv = gk[:].rearrange("p (j w) -> p j w", w=WT)
                    for j in range(BJ):
                        o = (b * 3 + k) * BJ + j
                        nc.gpsimd.indirect_dma_start(
                            out=gkv[:, j, :],
                            out_offset=None,
                            in_=t2f[b + 1][:],
                            in_offset=bass.IndirectOffsetOnAxis(
                                ap=bidx[:, o:o + 1], axis=0))
                    gs.append(gkv)
                prod = small.tile([128, BJ * E], f32, tag="prod", name="prod")
                pv = prod[:].rearrange("p (j e) -> p j e", e=E)
                sco = small.tile([128, 2 * BJ], f32, tag="sco", name="sco")
                for k in range(2):
                    nc.vector.tensor_tensor(out=pv, in0=gs[0][:, :, E:2 * E],
                                            in1=gs[k + 1][:, :, E:2 * E],
                                            op=ALU.mult)
                    nc.vector.tensor_reduce(
                        out=sco[:, k * BJ:(k + 1) * BJ], in_=pv,
                        axis=mybir.AxisListType.X, op=ALU.add)
                dd = small.tile([128, BJ], f32, tag="dd", name="dd")
                nc.vector.tensor_tensor(out=dd[:], in0=sco[:, 0:BJ],
                                        in1=sco[:, BJ:2 * BJ],
                                        op=ALU.subtract)
                aab = small.tile([128, BJ], f32, tag="aab", name="aab")
                nc.vector.tensor_scalar(out=aab[:], in0=dd[:], scalar1=-1.0,
                                        scalar2=None, op0=ALU.mult)
                nc.vector.tensor_tensor(out=aab[:], in0=aab[:], in1=dd[:],
                                        op=ALU.max)
                zex = small.tile([128, BJ], f32, tag="zex", name="zex")
                nc.scalar.activation(out=zex[:], in_=aab[:], func=AF.Exp,
                                     scale=-1.0)
                pol = small.tile([128, BJ], f32, tag="pol", name="pol")
                nc.vector.tensor_scalar(out=pol[:], in0=zex[:],
                                        scalar1=LOG1P_C[10],
                                        scalar2=LOG1P_C[9],
                                        op0=ALU.mult, op1=ALU.add)
                for k in range(8, -1, -1):
                    nc.vector.tensor_tensor(out=pol[:], in0=pol[:],
                                            in1=zex[:], op=ALU.mult)
                    nc.vector.tensor_scalar(out=pol[:], in0=pol[:],
                                            scalar1=LOG1P_C[k], scalar2=None,
                                            op0=ALU.add)
                nc.vector.tensor_scalar(out=dd[:], in0=dd[:], scalar1=-1.0,
                                        scalar2=0.0, op0=ALU.mult,
                                        op1=ALU.max)
                nc.vector.tensor_tensor(out=pol[:], in0=pol[:], in1=dd[:],
                                        op=ALU.add)
                nc.vector.tensor_reduce(out=blacc[:, b:b + 1], in_=pol[:],
                                        axis=mybir.AxisListType.X,
                                        op=ALU.add)

            # ================= program =================
            deg_pass(0)
            assemble(0)
            main_pass(0)
            deg_pass(1)
            assemble(1)
            loss_pass(0)
            main_pass(1)
            deg_pass(2)
            assemble(2)
            loss_pass(1)
            main_pass(2)
            assemble(3)
            loss_pass(2)

            # ---------------- final combine ----------------
            pack = small.tile([128, 2], f32, tag="pack", name="pack")
            nc.vector.tensor_reduce(out=pack[:, 0:1], in_=blacc[:],
                                    axis=mybir.AxisListType.X, op=ALU.add)
            nc.vector.tensor_reduce(out=pack[:, 1:2], in_=racc[:, :NREG],
                                    axis=mybir.AxisListType.X, op=ALU.add)
            fin_ps = ppy.tile([1, 2], f32, tag="fin", name="fin", bufs=1)
            nc.tensor.matmul(out=fin_ps[:], lhsT=onesf[:], rhs=pack[:],
                             start=True, stop=True)
            fin = small.tile([1, 2], f32, tag="fins", name="fins")
            nc.vector.tensor_copy(out=fin[:], in_=fin_ps[:])
            nc.sync.dma_start(out=lag_i[:], in_=fin[:])
            nc.gpsimd.collective_compute(
                "AllGather", ALU.bypass,
                replica_groups=[list(range(ncores))],
                ins=[lag_i[:].opt()], outs=[lag_o[:].opt()])
            lsb = small.tile([1, 2 * ncores], f32, tag="lsb", name="lsb")
            nc.sync.dma_start(
                out=lsb[:],
                in_=lag_o[:].rearrange("(o a) b -> o (a b)", o=1))
            bl = small.tile([1, 2], f32, tag="bl", name="bl")
            lv = lsb[:].rearrange("p (a b) -> p a b", b=2)
            nc.vector.tensor_reduce(out=bl[:, 0:1], in_=lv[:, :, 0:1],
                                    axis=mybir.AxisListType.XY, op=ALU.add)
            nc.vector.tensor_reduce(out=bl[:, 1:2], in_=lv[:, :, 1:2],
                                    axis=mybir.AxisListType.XY, op=ALU.add)
            res = small.tile([1, 1], f32, tag="res", name="res")
            nc.vector.tensor_scalar(out=res[:], in0=bl[:, 1:2],
                                    scalar1=cfg["reg_weight"] * 0.5,
                                    scalar2=None, op0=ALU.mult)
            nc.vector.tensor_tensor(out=res[:], in0=res[:], in1=bl[:, 0:1],
                                    op=ALU.add)
            nc.vector.tensor_scalar(out=res[:], in0=res[:],
                                    scalar1=1.0 / cfg["batch"],
                                    scalar2=None, op0=ALU.mult)
            nc.sync.dma_start(out=loss_out, in_=res[:])

    nc.compile()
    return nc


# ---------------------------------------------------------------------------
# Entry point
# ---------------------------------------------------------------------------
LAST_RESULTS = None


def kernel(**inputs) -> np.ndarray:
    global LAST_RESULTS
    cfg = FULL_CFG
    edges = np.asarray(inputs["edges"])
    sched, cols_arr, idx_arr = make_schedule_and_arrays(edges, cfg)
    in_maps = make_inputs_per_core(inputs, cfg, (sched, cols_arr, idx_arr))
    nc = build_program(cfg, sched)

    import os
    os.environ["BASS_NEVER_TRACE"] = "1"  # axon NTFF hook absent here
    from concourse import bass_utils
    res = bass_utils.run_bass_kernel_spmd(
        nc, in_maps, core_ids=list(range(cfg["ncores"])))
    LAST_RESULTS = res
    out = res.results[0]["loss"]
    return np.float32(out.reshape(-1)[0])



# revision 6
# speedup vs baseline: 2.0395x; 2.0395x over previous

# CRGCN multi-behavior GCN forward loss on 8 Trainium2 NeuronCores.
#
# Strategy (graph/data parallel, dest-node sharding):
#  - Nodes (users+items, 200000 -> padded 200704) are sharded row-wise across
#    8 cores (25088 = 196*128 nodes/core). Edges are partitioned by the shard
#    of their destination (col) node on the host, bucketed by (128-dest tile,
#    source bucket of 28672 rows) and padded so every 128-edge chunk maps to
#    one dest tile and one source bucket. The chunk schedule is the max over
#    cores so a single SPMD program fits all 8 cores.
#  - Per behavior each core holds a bf16 table T2 = [dinv*total | total]
#    ([200704, 128], 256B rows) for ALL nodes, produced by AllGather of
#    per-shard slabs. Message pass: dma_gather (int16 in-bucket indices) of
#    T2 rows for edge sources; a 0/1 one-hot (edge x dest-in-tile) built on
#    DVE from edge cols; PE matmul contracts edges, accumulating
#    S^T[feat, dest] = sum_e dinv[r_e]*total[r_e] x onehot in PSUM per dest
#    tile; then S @ W, *dinv[d], +b, l2-normalize, residual-accumulate into
#    the SBUF-resident fp32 total shard.
#  - deg (in-degree) is a one-hot x ones matmul (bf16, exact), per behavior,
#    from the same col data.
#  - BPR loss: batch rows sharded across cores; u/pos/neg rows fetched with
#    per-partition indirect DMA from the raw-total half of T2; dots +
#    softplus(-d) (relu + log1p poly) on-device; partials AllGathered so all
#    cores emit the identical final scalar.

import sys

sys.path.insert(0, "/opt/trn_rl_repo")

import dataclasses
import numpy as np

# ---------------- problem constants (hardcoded; kernel.py is standalone) ---
N_USERS = 100000
N_ITEMS = 100000
N_NODES = 200000
EMBED = 64
N_BEH = 3
BATCH = 4096
REG_WEIGHT = 1e-4
NCORES = 8

FULL_CFG = dict(
    ncores=NCORES,
    embed=EMBED,
    nbeh=N_BEH,
    shard=25088,          # 196 * 128
    nt=196,               # dest tiles per shard
    wt=128,               # T2 row width in bf16 elems (256B)
    nbuck=7,              # source buckets
    bucket=28672,         # rows per bucket (7 * 28672 = 200704)
    wtiles=8,             # dest tiles per gather window
    g=32,                 # chunks per one-hot build group
    flush=14,             # tiles per T2 staging flush (196 = 14*14)
    batch=BATCH,
    batch_per_core=BATCH // NCORES,   # 512
    n_nodes=N_NODES,
    reg_weight=REG_WEIGHT,
)


# ---------------------------------------------------------------------------
# Host-side preprocessing
# ---------------------------------------------------------------------------
def make_schedule_and_arrays(edges, cfg):
    """edges: [NB, 2, E]. Builds the (window, bucket, tile)-ordered common
    chunk schedule and the per-core col/idx arrays."""
    ncores = cfg["ncores"]
    NT = cfg["nt"]
    NB = cfg["nbeh"]
    NBK = cfg["nbuck"]
    BUCK = cfg["bucket"]
    WT = cfg["wtiles"]
    NW = (NT + WT - 1) // WT

    sched = {"C": [], "cells": [], "tiles": [], "windows": [],
             "tile_cstart": []}
    cols_arr = [[None] * NB for _ in range(ncores)]
    idx_arr = [[None] * NB for _ in range(ncores)]

    for b in range(NB):
        row = np.asarray(edges[b, 0], dtype=np.int64)
        col = np.asarray(edges[b, 1], dtype=np.int64)
        gt = col >> 7                       # global dest tile
        s_of = gt // NT                     # owning core
        t_of = gt - s_of * NT               # local dest tile
        beta = row // BUCK                  # source bucket
        # per (core, tile, bucket) counts
        cellkey = (s_of * NT + t_of) * NBK + beta
        cnt = np.bincount(cellkey, minlength=ncores * NT * NBK).reshape(
            ncores, NT, NBK)
        K_cell = -(-cnt.max(axis=0) // 128)           # [NT, NBK]
        empty_t = K_cell.sum(axis=1) == 0
        K_cell[empty_t, 0] = 1

        # gather order: (window, bucket, tile); consumption order:
        # (window, tile, bucket). Chunks get positions in both orders.
        C = int(K_cell.sum())
        cell_start = {}      # gather-order chunk start per cell
        cell_cstart = {}     # consumption-order chunk start per cell
        pos = 0
        for w in range(NW):
            ts = range(w * WT, min((w + 1) * WT, NT))
            for be in range(NBK):
                for t in ts:
                    if K_cell[t, be]:
                        cell_start[(t, be)] = pos
                        pos += int(K_cell[t, be])
        assert pos == C
        cpos = 0
        tile_cstart = np.zeros(NT + 1, dtype=np.int64)
        for w in range(NW):
            ts = range(w * WT, min((w + 1) * WT, NT))
            for t in ts:
                tile_cstart[t] = cpos
                for be in range(NBK):
                    if K_cell[t, be]:
                        cell_cstart[(t, be)] = cpos
                        cpos += int(K_cell[t, be])
        tile_cstart[NT] = cpos
        assert cpos == C

        # per-tile consumption: ordered chunk positions + total K per tile
        tiles = []
        for t in range(NT):
            plist = []
            for be in range(NBK):
                if K_cell[t, be]:
                    st = cell_start[(t, be)]
                    plist.extend(range(st, st + int(K_cell[t, be])))
            tiles.append(plist)

        # per-window gather segments: (bucket, pos_start, n_chunks)
        windows = []
        for w in range(NW):
            ts = range(w * WT, min((w + 1) * WT, NT))
            segs = []
            for be in range(NBK):
                n = int(sum(K_cell[t, be] for t in ts))
                if n:
                    st = min(cell_start[(t, be)] for t in ts
                             if K_cell[t, be])
                    segs.append((be, st, n))
            windows.append(segs)

        sched["C"].append(C)
        sched["cells"].append((K_cell, cell_start))
        sched["tiles"].append(tiles)
        sched["windows"].append(windows)
        sched["tile_cstart"].append(tile_cstart)

        # ------------- per-core arrays -------------
        starts_np = np.zeros((NT, NBK), dtype=np.int64)
        for (t, be), st in cell_start.items():
            starts_np[t, be] = st
        cstarts_np = np.zeros((NT, NBK), dtype=np.int64)
        for (t, be), st in cell_cstart.items():
            cstarts_np[t, be] = st
        for s in range(ncores):
            colv = np.full(C * 128, 128.0, dtype=np.float32)
            rowv = np.zeros(C * 128, dtype=np.int64)   # in-bucket idx
            sel = s_of == s
            r_s = row[sel]
            c_s = col[sel]
            t_s = t_of[sel]
            be_s = beta[sel]
            key = t_s * NBK + be_s
            order = np.argsort(key, kind="stable")
            r_s, c_s, t_s, be_s, key = (r_s[order], c_s[order], t_s[order],
                                        be_s[order], key[order])
            seg_start = np.searchsorted(key, np.arange(NT * NBK))
            within = np.arange(len(key)) - seg_start[key]
            dst = starts_np[t_s, be_s] * 128 + within
            cdst = cstarts_np[t_s, be_s] * 128 + within
            colv[cdst] = (c_s & 127).astype(np.float32)
            rowv[dst] = r_s - be_s * BUCK
            import ml_dtypes as _md
            cols_arr[s][b] = np.ascontiguousarray(
                colv.reshape(C, 128).T).astype(_md.bfloat16)   # [128, C]
            # idx16: [128, C*8]; gather element i -> [i%16 (+16k), off+i//16]
            iv = rowv.reshape(C * 128)
            i16 = np.zeros((16, C * 8), dtype=np.int16)
            ii = np.arange(C * 128)
            i16[ii % 16, ii // 16] = iv.astype(np.int16)
            idx_arr[s][b] = np.ascontiguousarray(np.tile(i16, (8, 1)))

    return sched, cols_arr, idx_arr


def make_inputs_per_core(inputs, cfg, sched_arrays):
    import ml_dtypes

    ncores = cfg["ncores"]
    SH = cfg["shard"]
    E = cfg["embed"]
    NB = cfg["nbeh"]
    BPC = cfg["batch_per_core"]
    BJ = BPC // 128
    n_nodes = cfg["n_nodes"]
    n_users = n_nodes // 2

    sched, cols_arr, idx_arr = sched_arrays

    user_emb = np.asarray(inputs["user_emb"], dtype=np.float32)
    item_emb = np.asarray(inputs["item_emb"], dtype=np.float32)
    gcn_weight = np.asarray(inputs["gcn_weight"], dtype=np.float32)
    gcn_bias = np.asarray(inputs["gcn_bias"], dtype=np.float32)
    batch_data = np.asarray(inputs["batch_data"], dtype=np.int64)

    total0 = np.concatenate([user_emb, item_emb], axis=0)

    iota = np.tile(np.arange(128, dtype=np.float32)[None, :],
                   (128, 1)).astype(ml_dtypes.bfloat16)
    w_bf = gcn_weight.astype(ml_dtypes.bfloat16)
    bb = np.tile(gcn_bias[:, None, :], (1, 128, 1)).astype(np.float32)

    in_maps = []
    for s in range(ncores):
        lo = s * SH
        hi = min((s + 1) * SH, n_nodes)
        init_shard = np.zeros((SH, E), dtype=np.float32)
        if hi > lo:
            init_shard[: hi - lo] = total0[lo:hi]

        bidx = np.zeros((NB * 3, 128, BJ), dtype=np.int32)
        rs = slice(s * BPC, (s + 1) * BPC)
        for b in range(NB):
            u = batch_data[rs, b, 0].astype(np.int32)
            p = batch_data[rs, b, 1].astype(np.int32) + n_users
            n = batch_data[rs, b, 2].astype(np.int32) + n_users
            for k, v in enumerate((u, p, n)):
                bidx[b * 3 + k] = v.reshape(BJ, 128).T

        m = {
            "init_shard": init_shard,
            "iota_in": iota,
            "w_in": w_bf,
            "bb_in": bb,
            "bidx_in": bidx,
        }
        for b in range(NB):
            m[f"col{b}"] = cols_arr[s][b]
            m[f"idx{b}"] = idx_arr[s][b]
        in_maps.append(m)
    return in_maps


# ---------------------------------------------------------------------------
# Device program
# ---------------------------------------------------------------------------
def build_program(cfg, sched, sim=False):
    from concourse import bass, bacc, mybir, tile

    dt = mybir.dt
    AF = mybir.ActivationFunctionType
    ALU = mybir.AluOpType

    ncores = cfg["ncores"]
    NT = cfg["nt"]
    SH = cfg["shard"]
    NTOT = SH * ncores
    E = cfg["embed"]
    WT = cfg["wt"]            # 128 table cols
    NBK = cfg["nbuck"]
    BUCK = cfg["bucket"]
    WTL = cfg["wtiles"]
    G = cfg["g"]
    FLUSH = cfg["flush"]
    BPC = cfg["batch_per_core"]
    BJ = BPC // 128
    NB = cfg["nbeh"]
    NV = NB + 1
    NW = (NT + WTL - 1) // WTL

    C = sched["C"]
    tiles_md = sched["tiles"]
    windows_md = sched["windows"]
    cstart_md = sched["tile_cstart"]

    # max chunks in any window (for the staging tile size)
    wch_max = 0
    for b in range(NB):
        for w in range(NW):
            wch = sum(n for (_, _, n) in windows_md[b][w])
            wch_max = max(wch_max, wch)

    def bc(ap, where, n):
        newap = list(ap.ap)
        newap.insert(where, [0, n])
        return dataclasses.replace(ap, ap=newap)

    nc = bacc.Bacc("TRN2", target_bir_lowering=False, debug=False,
                   num_devices=1 if sim else ncores, num_swdge_queues=4)

    def all_gather(src_tile, dst_tile, nrep):
        # sim mode: stand in for the collective with local HBM->HBM copies
        # of the same receive volume so TimelineSim can run (single-core,
        # no collectives) with comparable DMA load + dependencies.
        if sim:
            n = src_tile.shape[0]
            for r in range(nrep):
                nc.sync.dma_start(out=dst_tile[r * n:(r + 1) * n, :],
                                  in_=src_tile[:])
        else:
            nc.gpsimd.collective_compute(
                "AllGather", mybir.AluOpType.bypass,
                replica_groups=[list(range(nrep))],
                ins=[src_tile[:].opt()], outs=[dst_tile[:].opt()])

    f32, bf16, i32, i16 = dt.float32, dt.bfloat16, dt.int32, dt.int16
    shared = "Local"

    init_in = nc.dram_tensor("init_shard", [SH, E], f32,
                             kind="ExternalInput").ap()
    iota_in = nc.dram_tensor("iota_in", [128, 128], bf16,
                             kind="ExternalInput").ap()
    w_in = nc.dram_tensor("w_in", [NB, E, E], bf16, kind="ExternalInput").ap()
    bb_in = nc.dram_tensor("bb_in", [NB, 128, E], f32,
                           kind="ExternalInput").ap()
    bidx_in = nc.dram_tensor("bidx_in", [NB * 3, 128, BJ], i32,
                             kind="ExternalInput").ap()
    col_in = [nc.dram_tensor(f"col{b}", [128, C[b]], bf16,
                             kind="ExternalInput").ap() for b in range(NB)]
    idx_in = [nc.dram_tensor(f"idx{b}", [128, C[b] * 8], i16,
                             kind="ExternalInput").ap() for b in range(NB)]
    loss_out = nc.dram_tensor("loss", [1, 1], f32, kind="ExternalOutput").ap()

    with tile.TileContext(nc) as tc:
        with (
            tc.tile_pool(name="dram", bufs=1, space="DRAM") as dpool,
            tc.tile_pool(name="pers", bufs=1) as pers,
            tc.tile_pool(name="work", bufs=2) as work,
            tc.tile_pool(name="small", bufs=4) as small,
            tc.tile_pool(name="ppx", bufs=2, space="PSUM") as ppx,
            tc.tile_pool(name="ppy", bufs=2, space="PSUM") as ppy,
            tc.tile_pool(name="ppd", bufs=2, space="PSUM") as ppd,
        ):
            t2s = [dpool.tile([SH, WT], bf16, tag=f"t2s{v}",
                              name=f"t2s{v}") for v in range(NV)]
            t2f = [dpool.tile([NTOT, WT], bf16, tag=f"t2f{v}",
                              name=f"t2f{v}", addr_space=shared)
                   for v in range(NV)]
            lag_i = dpool.tile([1, 2], f32, tag="lag_i", name="lag_i")
            lag_o = dpool.tile([ncores, 2], f32, tag="lag_o", name="lag_o",
                               addr_space=shared)

            tot = pers.tile([128, NT * E], f32, tag="tot", name="tot")
            iota = pers.tile([128, 128], bf16, tag="iota", name="iota")
            wsb = pers.tile([E, NB * E], bf16, tag="wsb", name="wsb")
            bbsb = pers.tile([128, NB * E], f32, tag="bbsb", name="bbsb")
            bidx = pers.tile([128, NB * 3 * BJ], i32, tag="bidx", name="bidx")
            deg = [pers.tile([128, NT], f32, tag=f"deg{b}", name=f"deg{b}")
                   for b in range(NB)]
            dinv = [pers.tile([128, NT], f32, tag=f"dinv{v}", name=f"dinv{v}")
                    for v in range(NV)]
            onesb = pers.tile([128, 1], bf16, tag="onesb", name="onesb")
            onesf = pers.tile([128, 1], f32, tag="onesf", name="onesf")
            racc = pers.tile([128, 16], f32, tag="racc", name="racc")
            blacc = pers.tile([128, NB], f32, tag="blacc", name="blacc")

            nc.sync.dma_start(out=iota[:], in_=iota_in)
            nc.sync.dma_start(
                out=wsb[:].rearrange("k (b e) -> k b e", b=NB),
                in_=w_in.rearrange("b k e -> k b e"))
            nc.sync.dma_start(
                out=bbsb[:].rearrange("p (b e) -> p b e", b=NB),
                in_=bb_in.rearrange("b p e -> p b e"))
            nc.sync.dma_start(
                out=bidx[:].rearrange("p (a j) -> p a j", a=NB * 3),
                in_=bidx_in.rearrange("a p j -> p a j"))
            nc.sync.dma_start(
                out=tot[:].rearrange("p (t e) -> p t e", e=E),
                in_=init_in.rearrange("(t p) e -> p t e", p=128))
            nc.vector.memset(onesb[:], 1.0)
            nc.vector.memset(onesf[:], 1.0)
            nc.vector.memset(dinv[NB][:], 0.0)

            # reg term: sum of squares of the initial embeddings
            NREG = (NT * E + 1023) // 1024
            sqd = pers.tile([128, 1024], f32, tag="sqd", name="sqd")
            for i in range(NREG):
                sl = slice(i * 1024, min((i + 1) * 1024, NT * E))
                nc.scalar.activation(out=sqd[:, : sl.stop - sl.start],
                                     in_=tot[:, sl], func=AF.Square,
                                     accum_out=racc[:, i:i + 1])

            # ------- lazy consumption-ordered one-hot group builder -------
            class IndBuilder:
                """Builds one-hot groups for consumption positions
                [c0, c0+wch) on demand, in order, so only a few groups are
                live at once."""

                def __init__(self, cs, c0, wch):
                    self.cs, self.c0, self.wch = cs, c0, wch
                    self.groups = {}

                def get(self, cpos):
                    rel = cpos - self.c0
                    g0 = (rel // G) * G
                    if g0 not in self.groups:
                        gw = min(G, self.wch - g0)
                        ind = work.tile([128, G * 128], dt.bfloat16,
                                        tag="ind", name="ind", bufs=3)
                        iv = ind[:].rearrange("p (g d) -> p g d", d=128)
                        nc.vector.tensor_tensor(
                            out=iv[:, :gw, :],
                            in0=bc(iota[:], 1, gw),
                            in1=bc(self.cs[:, g0:g0 + gw], 2, 128),
                            op=ALU.is_equal)
                        self.groups[g0] = ind
                    return self.groups[g0], rel - g0

            # ---------------- deg pass ----------------
            def deg_pass(b):
                for w in range(NW):
                    segs = windows_md[b][w]
                    if not segs:
                        continue
                    wch = sum(n for (_, _, n) in segs)
                    t0w = w * WTL
                    c0 = int(cstart_md[b][t0w])
                    cs = small.tile([128, wch_max], bf16, tag="csd",
                                    name="csd", bufs=3)
                    nc.sync.dma_start(out=cs[:, :wch],
                                      in_=col_in[b][:, c0:c0 + wch])
                    bld = IndBuilder(cs, c0, wch)
                    for t in range(t0w, min((t0w + WTL), NT)):
                        plist = tiles_md[b][t]
                        dg_ps = ppd.tile([128, 1], f32, tag="dg", name="dg",
                                         bufs=3)
                        for j in range(len(plist)):
                            ind, r = bld.get(int(cstart_md[b][t]) + j)
                            nc.tensor.matmul(
                                out=dg_ps[:],
                                lhsT=ind[:, r * 128:(r + 1) * 128],
                                rhs=onesb[:],
                                start=(j == 0), stop=(j == len(plist) - 1))
                        nc.scalar.copy(out=deg[b][:, t:t + 1], in_=dg_ps[:])
                d1 = small.tile([128, NT], f32, tag="d1", name="d1")
                d2 = small.tile([128, NT], f32, tag="d2", name="d2")
                nc.vector.tensor_scalar(out=d1[:], in0=deg[b][:], scalar1=1.0,
                                        scalar2=None, op0=ALU.max)
                nc.scalar.activation(out=d1[:], in_=d1[:], func=AF.Sqrt)
                nc.vector.reciprocal(out=d1[:], in_=d1[:])
                nc.vector.tensor_scalar(out=d2[:], in0=deg[b][:], scalar1=0.0,
                                        scalar2=None, op0=ALU.is_gt)
                nc.vector.tensor_tensor(out=dinv[b][:], in0=d1[:], in1=d2[:],
                                        op=ALU.mult)

            # ------------- T2 assemble sweep + AllGather -------------
            def assemble(v):
                for tf in range(0, NT, FLUSH):
                    n = min(FLUSH, NT - tf)
                    s65 = work.tile([128, FLUSH * WT], bf16, tag="s65",
                                    name="s65")
                    for i in range(n):
                        t = tf + i
                        totsl = tot[:, t * E:(t + 1) * E]
                        nc.vector.tensor_scalar(
                            out=s65[:, i * WT:i * WT + E], in0=totsl,
                            scalar1=dinv[v][:, t:t + 1], scalar2=None,
                            op0=ALU.mult)
                        nc.scalar.copy(
                            out=s65[:, i * WT + E:i * WT + 2 * E], in_=totsl)
                    nc.sync.dma_start(
                        out=t2s[v][:].rearrange(
                            "(t p) w -> p t w", p=128)[:, tf:tf + n, :],
                        in_=s65[:, :n * WT].rearrange(
                            "p (t w) -> p t w", w=WT))
                all_gather(t2s[v], t2f[v], ncores)

            # ---------------- main pass ----------------
            def main_pass(b):
                for w in range(NW):
                    segs = windows_md[b][w]
                    g0 = min(st for (_, st, _) in segs)
                    wch = sum(n for (_, _, n) in segs)
                    t0w = w * WTL
                    c0 = int(cstart_md[b][t0w])
                    cs = small.tile([128, wch_max], bf16, tag="cs", name="cs",
                                    bufs=3)
                    nc.sync.dma_start(out=cs[:, :wch],
                                      in_=col_in[b][:, c0:c0 + wch])
                    ixs = small.tile([128, wch_max * 8], i16, tag="ixs",
                                     name="ixs", bufs=2)
                    nc.sync.dma_start(out=ixs[:, :wch * 8],
                                      in_=idx_in[b][:, g0 * 8:(g0 + wch) * 8])
                    gat = work.tile([128, wch_max * 128], bf16, tag="gat",
                                    name="gat")
                    gv = gat[:].rearrange("p (c e) -> p c e", e=128)
                    for (be, st, n) in segs:
                        for o in range(0, n, 8):
                            m = min(8, n - o)
                            so = st - g0 + o
                            nc.gpsimd.dma_gather(
                                out_ap=gv[:, so:so + m, :],
                                in_ap=t2f[b][be * BUCK:(be + 1) * BUCK, :],
                                idxs_ap=ixs[:, so * 8:(so + m) * 8],
                                num_idxs=m * 128,
                                num_idxs_reg=m * 128,
                                elem_size=WT)
                    bld = IndBuilder(cs, c0, wch)
                    for t in range(t0w, min(t0w + WTL, NT)):
                        plist = tiles_md[b][t]
                        xt_ps = ppx.tile([E, 128], f32, tag="xt", name="xt")
                        for j, pos in enumerate(plist):
                            ind, r = bld.get(int(cstart_md[b][t]) + j)
                            nc.tensor.matmul(
                                out=xt_ps[:],
                                lhsT=gv[:, pos - g0, 0:E],
                                rhs=ind[:, r * 128:(r + 1) * 128],
                                start=(j == 0), stop=(j == len(plist) - 1))
                        post_tile(b, t, xt_ps)

            def post_tile(b, t, xt_ps):
                xts = small.tile([E, 128], bf16, tag="xts", name="xts")
                nc.vector.tensor_copy(out=xts[:], in_=xt_ps[:])
                y_ps = ppy.tile([128, E], f32, tag="y", name="y")
                nc.tensor.matmul(out=y_ps[:], lhsT=xts[:],
                                 rhs=wsb[:, b * E:(b + 1) * E],
                                 start=True, stop=True)
                z = small.tile([128, E], f32, tag="z", name="z")
                nc.vector.tensor_scalar(out=z[:], in0=y_ps[:],
                                        scalar1=dinv[b][:, t:t + 1],
                                        scalar2=None, op0=ALU.mult)
                nc.vector.tensor_tensor(out=z[:], in0=z[:],
                                        in1=bbsb[:, b * E:(b + 1) * E],
                                        op=ALU.add)
                sq = small.tile([128, E], f32, tag="sq", name="sq")
                ss = small.tile([128, 1], f32, tag="ss", name="ss")
                nc.scalar.activation(out=sq[:], in_=z[:], func=AF.Square,
                                     accum_out=ss[:])
                nc.scalar.activation(out=ss[:], in_=ss[:], func=AF.Sqrt)
                nc.vector.tensor_scalar(out=ss[:], in0=ss[:], scalar1=1e-12,
                                        scalar2=None, op0=ALU.max)
                rin = small.tile([128, 1], f32, tag="rin", name="rin")
                nc.vector.reciprocal(out=rin[:], in_=ss[:])
                nc.vector.tensor_scalar(out=z[:], in0=z[:], scalar1=rin[:, :],
                                        scalar2=None, op0=ALU.mult)
                totsl = tot[:, t * E:(t + 1) * E]
                nc.vector.tensor_tensor(out=totsl, in0=totsl, in1=z[:],
                                        op=ALU.add)

            # ---------------- loss ----------------
            LOG1P_C = [2.4139025189026897e-09, 0.9999996692324197,
                       -0.499988759640371, 0.3331669190104936,
                       -0.2486582066434577, 0.19337637102999028,
                       -0.14517645896753417, 0.09470379566439587,
                       -0.04713346504062944, 0.015145372148722138,
                       -0.002288060381570317]

            def loss_pass(b):
                gs = []
                for k in range(3):
                    gk = small.tile([128, BJ * WT], bf16, tag=f"bg{k}",
                                    name=f"bg{k}")
                    gkv = gk[:].rearrange("p (j w) -> p j w", w=WT)
                    for j in range(BJ):
                        o = (b * 3 + k) * BJ + j
                        nc.gpsimd.indirect_dma_start(
                            out=gkv[:, j, :],
                            out_offset=None,
                            in_=t2f[b + 1][:],
                            in_offset=bass.IndirectOffsetOnAxis(
                                ap=bidx[:, o:o + 1], axis=0))
                    gs.append(gkv)
                prod = small.tile([128, BJ * E], f32, tag="prod", name="prod")
                pv = prod[:].rearrange("p (j e) -> p j e", e=E)
                sco = small.tile([128, 2 * BJ], f32, tag="sco", name="sco")
                for k in range(2):
                    nc.vector.tensor_tensor(out=pv, in0=gs[0][:, :, E:2 * E],
                                            in1=gs[k + 1][:, :, E:2 * E],
                                            op=ALU.mult)
                    nc.vector.tensor_reduce(
                        out=sco[:, k * BJ:(k + 1) * BJ], in_=pv,
                        axis=mybir.AxisListType.X, op=ALU.add)
                dd = small.tile([128, BJ], f32, tag="dd", name="dd")
                nc.vector.tensor_tensor(out=dd[:], in0=sco[:, 0:BJ],
                                        in1=sco[:, BJ:2 * BJ],
                                        op=ALU.subtract)
                aab = small.tile([128, BJ], f32, tag="aab", name="aab")
                nc.vector.tensor_scalar(out=aab[:], in0=dd[:], scalar1=-1.0,
                                        scalar2=None, op0=ALU.mult)
                nc.vector.tensor_tensor(out=aab[:], in0=aab[:], in1=dd[:],
                                        op=ALU.max)
                zex = small.tile([128, BJ], f32, tag="zex", name="zex")
                nc.scalar.activation(out=zex[:], in_=aab[:], func=AF.Exp,
                                     scale=-1.0)
                pol = small.tile([128, BJ], f32, tag="pol", name="pol")
                nc.vector.tensor_scalar(out=pol[:], in0=zex[:],
                                        scalar1=LOG1P_C[10],
                                        scalar2=LOG1P_C[9],
                                        op0=ALU.mult, op1=ALU.add)
                for k in range(8, -1, -1):
                    nc.vector.tensor_tensor(out=pol[:], in0=pol[:],
                                            in1=zex[:], op=ALU.mult)
                    nc.vector.tensor_scalar(out=pol[:], in0=pol[:],
                                            scalar1=LOG1P_C[k], scalar2=None,
                                            op0=ALU.add)
                nc.vector.tensor_scalar(out=dd[:], in0=dd[:], scalar1=-1.0,
                                        scalar2=0.0, op0=ALU.mult,
                                        op1=ALU.max)
                nc.vector.tensor_tensor(out=pol[:], in0=pol[:], in1=dd[:],
                                        op=ALU.add)
                nc.vector.tensor_reduce(out=blacc[:, b:b + 1], in_=pol[:],
                                        axis=mybir.AxisListType.X,
                                        op=ALU.add)

            # ================= program =================
            deg_pass(0)
            assemble(0)
            main_pass(0)
            deg_pass(1)
            assemble(1)
            loss_pass(0)
            main_pass(1)
            deg_pass(2)
            assemble(2)
            loss_pass(1)
            main_pass(2)
            assemble(3)
            loss_pass(2)

            # ---------------- final combine ----------------
            pack = small.tile([128, 2], f32, tag="pack", name="pack")
            nc.vector.tensor_reduce(out=pack[:, 0:1], in_=blacc[:],
                                    axis=mybir.AxisListType.X, op=ALU.add)
            nc.vector.tensor_reduce(out=pack[:, 1:2], in_=racc[:, :NREG],
                                    axis=mybir.AxisListType.X, op=ALU.add)
            fin_ps = ppy.tile([1, 2], f32, tag="fin", name="fin", bufs=1)
            nc.tensor.matmul(out=fin_ps[:], lhsT=onesf[:], rhs=pack[:],
                             start=True, stop=True)
            fin = small.tile([1, 2], f32, tag="fins", name="fins")
            nc.vector.tensor_copy(out=fin[:], in_=fin_ps[:])
            nc.sync.dma_start(out=lag_i[:], in_=fin[:])
            all_gather(lag_i, lag_o, ncores)
            lsb = small.tile([1, 2 * ncores], f32, tag="lsb", name="lsb")
            nc.sync.dma_start(
                out=lsb[:],
                in_=lag_o[:].rearrange("(o a) b -> o (a b)", o=1))
            bl = small.tile([1, 2], f32, tag="bl", name="bl")
            lv = lsb[:].rearrange("p (a b) -> p a b", b=2)
            nc.vector.tensor_reduce(out=bl[:, 0:1], in_=lv[:, :, 0:1],
                                    axis=mybir.AxisListType.XY, op=ALU.add)
            nc.vector.tensor_reduce(out=bl[:, 1:2], in_=lv[:, :, 1:2],
                                    axis=mybir.AxisListType.XY, op=ALU.add)
            res = small.tile([1, 1], f32, tag="res", name="res")
            nc.vector.tensor_scalar(out=res[:], in0=bl[:, 1:2],
                                    scalar1=cfg["reg_weight"] * 0.5,
                                    scalar2=None, op0=ALU.mult)
            nc.vector.tensor_tensor(out=res[:], in0=res[:], in1=bl[:, 0:1],
                                    op=ALU.add)
            nc.vector.tensor_scalar(out=res[:], in0=res[:],
                                    scalar1=1.0 / cfg["batch"],
                                    scalar2=None, op0=ALU.mult)
            nc.sync.dma_start(out=loss_out, in_=res[:])

    nc.compile()
    return nc


# ---------------------------------------------------------------------------
# Entry point
# ---------------------------------------------------------------------------
LAST_RESULTS = None


def kernel(**inputs) -> np.ndarray:
    global LAST_RESULTS
    cfg = FULL_CFG
    edges = np.asarray(inputs["edges"])
    sched, cols_arr, idx_arr = make_schedule_and_arrays(edges, cfg)
    in_maps = make_inputs_per_core(inputs, cfg, (sched, cols_arr, idx_arr))
    nc = build_program(cfg, sched)

    import os
    os.environ["BASS_NEVER_TRACE"] = "1"  # axon NTFF hook absent here
    from concourse import bass_utils
    res = bass_utils.run_bass_kernel_spmd(
        nc, in_maps, core_ids=list(range(cfg["ncores"])))
    LAST_RESULTS = res
    out = res.results[0]["loss"]
    return np.float32(out.reshape(-1)[0])

